# revision 21
# baseline (speedup 1.0000x reference)
"""Trainium2 Bass kernel for LISTA (nn_LISTA_37976100831401).

Data-parallel sharding: batch 16384 -> 8 NeuronCores x 2048 rows.
We / S / theta are replicated on every core; no cross-device comm.

Per-core algorithm (all in the transposed [feature, batch] orientation):
  B.T  = We @ X.T                  (1024, 2048)
  Z0 = soft(B);  Z_{t+1} = soft(B + Z_t @ S.T), t = 0..15
  soft(x) = relu(x - theta) - relu(-x - theta) = x - clip(x, -theta, theta)

All matmul operands are bf16: the PE streams 1 column/cycle for both
bf16 and fp32r, but bf16 weights get fast-weight-load plus
background-buffer overlap, so the per-matmul LDWEIGHTS cost disappears
(fp32r measured 227ns vs 216ns per matmul issue gap).  Accumulation
stays fp32 in PSUM; intermediates (C, relu pair) and the stored Z/B
tiles are bf16, which halves DVE/ACT data movement and doubles DVE
throughput on the all-16-bit combine.  Validated end-to-end error
~0.006-0.008 * max|expected| vs the 2e-2 gate.

Engine balance per steady-state step tile (PE budget 8 matmuls ~1.73us):
  DVE:  ct = psum + B (bf16 out), z = af - df (16-bit, 2x rate)  ~1.0us
  ACT:  af = relu(ct - th), df = relu(-ct - th)                  ~1.1us
An all-DVE soft-threshold measured DVE at 95% busy and stalled the PE.

The B phase is the tricky part: its matmul groups are only K=256 (2
matmuls, 0.43us/tile) while the per-tile threshold+copy work is
~1.6us, so a straight b(0..3) emission stalls the PE for tens of us
(measured 49us at HAM K=4/8).  Fix: alternate the b-phase threshold
between the DVE-clip path and the ACT-relu path per tile, and pipeline
the b phases into the first step sweep:
  b0 b1 s1(0) b2 s1(1) b3 s1(2) s1(3) s2(0) ...
so the PE always has step matmuls to run while b-phase chains drain.

Layout: Z.T keeps the feature dim m=1024 on SBUF partitions (8 tiles
of 128) and batch on the free dim; C.T = S @ Z.T + B.T accumulates in
PSUM via psum[j,b] += ST[k][:, j128].T @ ZT[k][:, b] and the matmul
OUTPUT layout [j, b] equals the INPUT layout [k, b] of the next step,
so no transposes anywhere.  The last step stays in [j, b] too and DMAs
Z.T out; the host transposes (host marshaling is not on the device
critical path, same as the input-side X.T/We.T/S.T preparation).
"""

import time
from contextlib import ExitStack

import ml_dtypes
import numpy as np

import concourse.bacc as bacc
import concourse.mybir as mybir
import concourse.tile as tile
from concourse import bass_utils

FP32 = mybir.dt.float32
BF16 = mybir.dt.bfloat16
AL = mybir.AluOpType
AF = mybir.ActivationFunctionType

N_CORES = 8
B_TOTAL, N_IN, M = 16384, 256, 1024
B_CORE = B_TOTAL // N_CORES  # 2048
T_STEPS = 16                 # scan length in the reference
CHUNK = 512                  # batch columns per j-sweep (= PSUM bank cap)
N_CHUNKS = B_CORE // CHUNK   # 4
KT = M // 128                # 8 feature tiles of 128
NT = N_IN // 128             # 2 input-feature tiles


def _emit(ctx: ExitStack, tc: tile.TileContext, XT, WeT, ST, NTH, TH, Z):
    nc = tc.nc

    const_pool = ctx.enter_context(tc.tile_pool(name="const", bufs=1))
    psum_pool = ctx.enter_context(tc.tile_pool(name="psum", bufs=1, space="PSUM"))
    xt_pool = ctx.enter_context(tc.tile_pool(name="xt", bufs=1))
    bt_pool = ctx.enter_context(tc.tile_pool(name="bt", bufs=1))
    zt_pool = ctx.enter_context(tc.tile_pool(name="zt", bufs=1))
    tmp_pool = ctx.enter_context(tc.tile_pool(name="tmp", bufs=1))
    out_pool = ctx.enter_context(tc.tile_pool(name="zout", bufs=1))

    # ---- constants -------------------------------------------------------
    nth = const_pool.tile([128, KT], FP32, name="nth")
    th = const_pool.tile([128, KT], FP32, name="th")
    wet = [const_pool.tile([128, M], BF16, name=f"wet{nt}") for nt in range(NT)]
    st = [const_pool.tile([128, M], BF16, name=f"st{kt}") for kt in range(KT)]

    # ---- per-chunk state -------------------------------------------------
    xts = {}  # chunk -> [NT] tiles [128, CHUNK]   (X.T slab, bf16)
    bts = {}  # chunk -> [KT] tiles [128, CHUNK]   (B.T slab, bf16)
    zts = {}  # chunk -> [KT] tiles [128, CHUNK]   (current Z.T, bf16)

    def x_phase(c, dma_eng):
        xts[c] = [
            xt_pool.tile([128, CHUNK], BF16, name=f"xt{nt}", tag=f"xt{nt}", bufs=4)
            for nt in range(NT)
        ]
        for nt in range(NT):
            dma_eng.dma_start(
                xts[c][nt][:],
                XT[nt * 128 : (nt + 1) * 128, c * CHUNK : (c + 1) * CHUNK],
            )

    def emit_head_dmas():
        # Head critical path: the b phases need wet + xt(0..3) first; S.T
        # row-blocks are needed by step(0) (~19us in): spread over three
        # queues right behind the critical loads.
        nc.scalar.dma_start(nth[:], NTH)
        nc.scalar.dma_start(th[:], TH)
        for nt in range(NT):
            nc.sync.dma_start(wet[nt][:], WeT[nt * 128 : (nt + 1) * 128, :])
        x_phase(0, nc.scalar)
        x_phase(1, nc.gpsimd)
        engs = [nc.scalar, nc.sync, nc.gpsimd]
        for kt in range(KT):
            engs[kt % 3].dma_start(st[kt][:], ST[kt * 128 : (kt + 1) * 128, :])
        x_phase(2, nc.sync)
        x_phase(3, nc.gpsimd)

    def thresh_act(jt, src, out_dtype, out_pool_, out_tag, out_bufs, io_dtype):
        # relu pair on ACT, combine on DVE (2x rate when io_dtype is bf16)
        af = tmp_pool.tile([128, CHUNK], io_dtype, name="af", tag="af", bufs=3)
        nc.scalar.activation(
            af[:], src[:], AF.Relu, bias=nth[:, jt : jt + 1], scale=1.0
        )
        df = tmp_pool.tile([128, CHUNK], io_dtype, name="df", tag="df", bufs=3)
        nc.scalar.activation(
            df[:], src[:], AF.Relu, bias=nth[:, jt : jt + 1], scale=-1.0
        )
        z = out_pool_.tile(
            [128, CHUNK], out_dtype, name="z", tag=out_tag, bufs=out_bufs
        )
        nc.vector.tensor_sub(z[:], af[:], df[:])
        return z

    def thresh_dve(jt, src, out_dtype, out_pool_, out_tag, out_bufs):
        # clip + combine, both on DVE
        cl = tmp_pool.tile([128, CHUNK], FP32, name="cl", tag="cl", bufs=3)
        nc.vector.tensor_scalar(
            cl[:], src[:], th[:, jt : jt + 1], nth[:, jt : jt + 1],
            op0=AL.min, op1=AL.max,
        )
        z = out_pool_.tile(
            [128, CHUNK], out_dtype, name="z", tag=out_tag, bufs=out_bufs
        )
        nc.vector.tensor_sub(z[:], src[:], cl[:])
        return z

    def b_phase(c):
        # B.T = We @ X.T ; Z0 = soft(B), threshold straight from PSUM.
        # PE work here is light (2-matmul groups), so the threshold load
        # alternates DVE/ACT per tile and the B copies mostly ride ACT.
        bts[c] = []
        zts[c] = []
        for jt in range(KT):
            ps = psum_pool.tile([128, CHUNK], FP32, name="psb", tag="mm", bufs=6)
            for nt in range(NT):
                nc.tensor.matmul(
                    ps[:],
                    wet[nt][:, jt * 128 : (jt + 1) * 128],
                    xts[c][nt][:],
                    start=(nt == 0),
                    stop=(nt == NT - 1),
                )
            if jt % 2 == 0:
                z0 = thresh_act(jt, ps, BF16, zt_pool, f"zt{jt}", 6, BF16)
            else:
                z0 = thresh_dve(jt, ps, BF16, zt_pool, f"zt{jt}", 6)
            btile = bt_pool.tile(
                [128, CHUNK], BF16, name="btile", tag=f"bt{jt}", bufs=4
            )
            if jt % 4 == 1:
                nc.vector.tensor_copy(btile[:], ps[:])
            else:
                nc.scalar.copy(btile[:], ps[:])
            bts[c].append(btile)
            zts[c].append(z0)

    def step(c, final=False):
        # Z <- soft(B + Z @ S.T), in the [j, b] orientation.
        zcur = zts[c]
        znew = []
        for jt in range(KT):
            ps = psum_pool.tile([128, CHUNK], FP32, name="pss", tag="mm", bufs=6)
            for kt in range(KT):
                nc.tensor.matmul(
                    ps[:],
                    st[kt][:, jt * 128 : (jt + 1) * 128],
                    zcur[kt][:],
                    start=(kt == 0),
                    stop=(kt == KT - 1),
                )
            ct = tmp_pool.tile(
                [128, CHUNK], FP32 if final else BF16, name="ct", tag="ct", bufs=3
            )
            nc.vector.tensor_add(ct[:], ps[:], bts[c][jt][:])
            if final:
                zo = thresh_act(jt, ct, FP32, out_pool, "zo", 3, FP32)
                dma_eng = nc.sync if jt % 2 == 0 else nc.scalar
                dma_eng.dma_start(
                    Z[jt * 128 : (jt + 1) * 128, c * CHUNK : (c + 1) * CHUNK],
                    zo[:],
                )
            else:
                zn = thresh_act(jt, ct, BF16, zt_pool, f"zt{jt}", 6, BF16)
                znew.append(zn)
        zts[c] = znew

    emit_head_dmas()
    b_phase(0)
    b_phase(1)
    step(0)
    b_phase(2)
    step(1)
    b_phase(3)
    step(2)
    step(3)
    for _ in range(T_STEPS - 2):
        for c in range(N_CHUNKS):
            step(c)
    for c in range(N_CHUNKS):
        step(c, final=True)


def build_nc():
    nc = bacc.Bacc("TRN2", target_bir_lowering=False, debug=False)
    XT = nc.dram_tensor("XT", [N_IN, B_CORE], BF16, kind="ExternalInput")
    WeT = nc.dram_tensor("WeT", [N_IN, M], BF16, kind="ExternalInput")
    ST = nc.dram_tensor("ST", [M, M], BF16, kind="ExternalInput")
    NTH = nc.dram_tensor("NTH", [128, KT], FP32, kind="ExternalInput")
    TH = nc.dram_tensor("TH", [128, KT], FP32, kind="ExternalInput")
    Z = nc.dram_tensor("Z", [M, B_CORE], FP32, kind="ExternalOutput")
    with tile.TileContext(nc) as tc:
        with ExitStack() as ctx:
            _emit(ctx, tc, XT.ap(), WeT.ap(), ST.ap(), NTH.ap(), TH.ap(), Z.ap())
    nc.compile()
    return nc


_NC_CACHE = None


def _get_nc():
    global _NC_CACHE
    if _NC_CACHE is None:
        _NC_CACHE = build_nc()
    return _NC_CACHE


def make_in_maps(X, We, S, theta):
    X = np.asarray(X, dtype=np.float32)
    WeT = np.ascontiguousarray(
        np.asarray(We, dtype=np.float32).T.astype(ml_dtypes.bfloat16)
    )
    ST = np.ascontiguousarray(
        np.asarray(S, dtype=np.float32).T.astype(ml_dtypes.bfloat16)
    )
    TH = np.ascontiguousarray(
        np.asarray(theta, dtype=np.float32).reshape(KT, 128).T
    )
    return [
        {
            "XT": np.ascontiguousarray(
                X[i * B_CORE : (i + 1) * B_CORE].T.astype(ml_dtypes.bfloat16)
            ),
            "WeT": WeT,
            "ST": ST,
            "NTH": np.ascontiguousarray(-TH),
            "TH": TH,
        }
        for i in range(N_CORES)
    ]


def gather_out(results):
    return np.concatenate(
        [np.ascontiguousarray(results[i]["Z"].T) for i in range(N_CORES)],
        axis=0,
    ).astype(np.float32, copy=False)


def run(X, We, S, theta, trace=False, **trace_kwargs):
    nc = _get_nc()
    in_maps = make_in_maps(X, We, S, theta)
    # The PJRT compile callback can fail transiently ("CallFunctionObjArgs");
    # a retry in the same process succeeds.
    last_err = None
    for _attempt in range(3):
        try:
            res = bass_utils.run_bass_kernel_spmd(
                nc, in_maps, list(range(N_CORES)), trace=trace, **trace_kwargs
            )
            break
        except Exception as e:  # noqa: BLE001
            last_err = e
            time.sleep(2.0)
    else:
        raise last_err
    return gather_out(res.results), res


def kernel(X, We, S, theta):
    Z, _ = run(X, We, S, theta, trace=False)
    return Z


# revision 23
# speedup vs baseline: 1.0053x; 1.0053x over previous
"""Trainium2 Bass kernel for LISTA (nn_LISTA_37976100831401).

Data-parallel sharding: batch 16384 -> 8 NeuronCores x 2048 rows.
We / S / theta are replicated on every core; no cross-device comm.

Per-core algorithm (all in the transposed [feature, batch] orientation):
  B.T  = We @ X.T                  (1024, 2048)
  Z0 = soft(B);  Z_{t+1} = soft(B + Z_t @ S.T), t = 0..15
  soft(x) = relu(x - theta) - relu(-x - theta) = x - clip(x, -theta, theta)

All matmul operands are bf16: the PE streams 1 column/cycle for both
bf16 and fp32r, but bf16 weights get fast-weight-load plus
background-buffer overlap, so the per-matmul LDWEIGHTS cost disappears
(fp32r measured 227ns vs 216ns per matmul issue gap).  Accumulation
stays fp32 in PSUM; intermediates (C, relu pair) and the stored Z/B
tiles are bf16, which halves DVE/ACT data movement and doubles DVE
throughput on the all-16-bit combine.  Validated end-to-end error
~0.006-0.008 * max|expected| vs the 2e-2 gate.

Engine balance per steady-state step tile (PE budget 8 matmuls ~1.73us):
  DVE:  ct = psum + B (bf16 out), z = af - df (16-bit, 2x rate)  ~1.0us
  ACT:  af = relu(ct - th), df = relu(-ct - th)                  ~1.1us
An all-DVE soft-threshold measured DVE at 95% busy and stalled the PE.

The B phase is the tricky part: its matmul groups are only K=256 (2
matmuls, 0.43us/tile) while the per-tile threshold+copy work is
~1.6us, so a straight b(0..3) emission stalls the PE for tens of us
(measured 49us at HAM K=4/8).  Fix: alternate the b-phase threshold
between the DVE-clip path and the ACT-relu path per tile, and pipeline
the b phases into the first step sweep:
  b0 b1 s1(0) b2 s1(1) b3 s1(2) s1(3) s2(0) ...
so the PE always has step matmuls to run while b-phase chains drain.

Layout: Z.T keeps the feature dim m=1024 on SBUF partitions (8 tiles
of 128) and batch on the free dim; C.T = S @ Z.T + B.T accumulates in
PSUM via psum[j,b] += ST[k][:, j128].T @ ZT[k][:, b] and the matmul
OUTPUT layout [j, b] equals the INPUT layout [k, b] of the next step,
so no transposes anywhere.  The last step stays in [j, b] too and DMAs
Z.T out; the host transposes (host marshaling is not on the device
critical path, same as the input-side X.T/We.T/S.T preparation).
"""

import time
from contextlib import ExitStack

import ml_dtypes
import numpy as np

import concourse.bacc as bacc
import concourse.mybir as mybir
import concourse.tile as tile
from concourse import bass_utils

FP32 = mybir.dt.float32
BF16 = mybir.dt.bfloat16
AL = mybir.AluOpType
AF = mybir.ActivationFunctionType

N_CORES = 8
B_TOTAL, N_IN, M = 16384, 256, 1024
B_CORE = B_TOTAL // N_CORES  # 2048
T_STEPS = 16                 # scan length in the reference
CHUNK = 512                  # batch columns per j-sweep (= PSUM bank cap)
N_CHUNKS = B_CORE // CHUNK   # 4
KT = M // 128                # 8 feature tiles of 128
NT = N_IN // 128             # 2 input-feature tiles


def _emit(ctx: ExitStack, tc: tile.TileContext, XT, WeT, ST, NTH, TH, Z):
    nc = tc.nc

    const_pool = ctx.enter_context(tc.tile_pool(name="const", bufs=1))
    psum_pool = ctx.enter_context(tc.tile_pool(name="psum", bufs=1, space="PSUM"))
    xt_pool = ctx.enter_context(tc.tile_pool(name="xt", bufs=1))
    bt_pool = ctx.enter_context(tc.tile_pool(name="bt", bufs=1))
    zt_pool = ctx.enter_context(tc.tile_pool(name="zt", bufs=1))
    tmp_pool = ctx.enter_context(tc.tile_pool(name="tmp", bufs=1))
    out_pool = ctx.enter_context(tc.tile_pool(name="zout", bufs=1))

    # ---- constants -------------------------------------------------------
    nth = const_pool.tile([128, KT], FP32, name="nth")
    th = const_pool.tile([128, KT], FP32, name="th")
    wet = [const_pool.tile([128, M], BF16, name=f"wet{nt}") for nt in range(NT)]
    st = [const_pool.tile([128, M], BF16, name=f"st{kt}") for kt in range(KT)]

    # ---- per-chunk state -------------------------------------------------
    xts = {}  # chunk -> [NT] tiles [128, CHUNK]   (X.T slab, bf16)
    bts = {}  # chunk -> [KT] tiles [128, CHUNK]   (B.T slab, bf16)
    zts = {}  # chunk -> [KT] tiles [128, CHUNK]   (current Z.T, bf16)

    def x_phase(c, dma_eng):
        xts[c] = [
            xt_pool.tile([128, CHUNK], BF16, name=f"xt{nt}", tag=f"xt{nt}", bufs=4)
            for nt in range(NT)
        ]
        for nt in range(NT):
            dma_eng.dma_start(
                xts[c][nt][:],
                XT[nt * 128 : (nt + 1) * 128, c * CHUNK : (c + 1) * CHUNK],
            )

    def emit_head_dmas():
        # Head critical path: the b phases need wet + xt(0..3) first; S.T
        # row-blocks are needed by step(0) (~19us in): spread over three
        # queues right behind the critical loads.
        nc.scalar.dma_start(nth[:], NTH)
        nc.scalar.dma_start(th[:], TH)
        for nt in range(NT):
            nc.sync.dma_start(wet[nt][:], WeT[nt * 128 : (nt + 1) * 128, :])
        x_phase(0, nc.scalar)
        x_phase(1, nc.gpsimd)
        engs = [nc.scalar, nc.sync, nc.gpsimd]
        for kt in range(KT):
            engs[kt % 3].dma_start(st[kt][:], ST[kt * 128 : (kt + 1) * 128, :])
        x_phase(2, nc.sync)
        x_phase(3, nc.gpsimd)

    def thresh_act(jt, src, out_dtype, out_pool_, out_tag, out_bufs, io_dtype):
        # relu pair on ACT, combine on DVE (2x rate when io_dtype is bf16)
        af = tmp_pool.tile([128, CHUNK], io_dtype, name="af", tag="af", bufs=3)
        nc.scalar.activation(
            af[:], src[:], AF.Relu, bias=nth[:, jt : jt + 1], scale=1.0
        )
        df = tmp_pool.tile([128, CHUNK], io_dtype, name="df", tag="df", bufs=3)
        nc.scalar.activation(
            df[:], src[:], AF.Relu, bias=nth[:, jt : jt + 1], scale=-1.0
        )
        z = out_pool_.tile(
            [128, CHUNK], out_dtype, name="z", tag=out_tag, bufs=out_bufs
        )
        nc.vector.tensor_sub(z[:], af[:], df[:])
        return z

    def thresh_dve(jt, src, out_dtype, out_pool_, out_tag, out_bufs):
        # clip + combine, both on DVE
        cl = tmp_pool.tile([128, CHUNK], FP32, name="cl", tag="cl", bufs=3)
        nc.vector.tensor_scalar(
            cl[:], src[:], th[:, jt : jt + 1], nth[:, jt : jt + 1],
            op0=AL.min, op1=AL.max,
        )
        z = out_pool_.tile(
            [128, CHUNK], out_dtype, name="z", tag=out_tag, bufs=out_bufs
        )
        nc.vector.tensor_sub(z[:], src[:], cl[:])
        return z

    def b_phase(c):
        # B.T = We @ X.T ; Z0 = soft(B), threshold straight from PSUM.
        # PE work here is light (2-matmul groups), so the threshold load
        # alternates DVE/ACT per tile and the B copies mostly ride ACT.
        bts[c] = []
        zts[c] = []
        for jt in range(KT):
            ps = psum_pool.tile([128, CHUNK], FP32, name="psb", tag="mm", bufs=6)
            for nt in range(NT):
                nc.tensor.matmul(
                    ps[:],
                    wet[nt][:, jt * 128 : (jt + 1) * 128],
                    xts[c][nt][:],
                    start=(nt == 0),
                    stop=(nt == NT - 1),
                )
            if jt % 2 == 0:
                z0 = thresh_act(jt, ps, BF16, zt_pool, f"zt{jt}", 6, BF16)
            else:
                z0 = thresh_dve(jt, ps, BF16, zt_pool, f"zt{jt}", 6)
            btile = bt_pool.tile(
                [128, CHUNK], BF16, name="btile", tag=f"bt{jt}", bufs=4
            )
            if jt % 4 == 1:
                nc.vector.tensor_copy(btile[:], ps[:])
            else:
                nc.scalar.copy(btile[:], ps[:])
            bts[c].append(btile)
            zts[c].append(z0)

    def step(c, final=False):
        # Z <- soft(B + Z @ S.T), in the [j, b] orientation.
        zcur = zts[c]
        znew = []
        for jt in range(KT):
            ps = psum_pool.tile([128, CHUNK], FP32, name="pss", tag="mm", bufs=6)
            for kt in range(KT):
                nc.tensor.matmul(
                    ps[:],
                    st[kt][:, jt * 128 : (jt + 1) * 128],
                    zcur[kt][:],
                    start=(kt == 0),
                    stop=(kt == KT - 1),
                )
            ct = tmp_pool.tile(
                [128, CHUNK], FP32 if final else BF16, name="ct", tag="ct", bufs=3
            )
            nc.vector.tensor_add(ct[:], ps[:], bts[c][jt][:])
            if final:
                zo = thresh_act(jt, ct, FP32, out_pool, "zo", 3, FP32)
                dma_eng = nc.sync if jt % 2 == 0 else nc.scalar
                dma_eng.dma_start(
                    Z[jt * 128 : (jt + 1) * 128, c * CHUNK : (c + 1) * CHUNK],
                    zo[:],
                )
            else:
                zn = thresh_act(jt, ct, BF16, zt_pool, f"zt{jt}", 6, BF16)
                znew.append(zn)
        zts[c] = znew

    emit_head_dmas()
    b_phase(0)
    b_phase(1)
    step(0)
    b_phase(2)
    step(1)
    b_phase(3)
    step(2)
    step(3)
    for _ in range(T_STEPS - 2):
        for c in range(N_CHUNKS):
            step(c)
    for c in range(N_CHUNKS):
        step(c, final=True)


def build_nc():
    nc = bacc.Bacc("TRN2", target_bir_lowering=False, debug=False)
    XT = nc.dram_tensor("XT", [N_IN, B_CORE], BF16, kind="ExternalInput")
    WeT = nc.dram_tensor("WeT", [N_IN, M], BF16, kind="ExternalInput")
    ST = nc.dram_tensor("ST", [M, M], BF16, kind="ExternalInput")
    NTH = nc.dram_tensor("NTH", [128, KT], FP32, kind="ExternalInput")
    TH = nc.dram_tensor("TH", [128, KT], FP32, kind="ExternalInput")
    Z = nc.dram_tensor("Z", [M, B_CORE], FP32, kind="ExternalOutput")
    with tile.TileContext(nc) as tc:
        with ExitStack() as ctx:
            _emit(ctx, tc, XT.ap(), WeT.ap(), ST.ap(), NTH.ap(), TH.ap(), Z.ap())
    nc.compile()
    return nc


_NC_CACHE = None


def _get_nc():
    global _NC_CACHE
    if _NC_CACHE is None:
        _NC_CACHE = build_nc()
    return _NC_CACHE


def make_in_maps(X, We, S, theta):
    X = np.asarray(X, dtype=np.float32)
    WeT = np.ascontiguousarray(
        np.asarray(We, dtype=np.float32).T.astype(ml_dtypes.bfloat16)
    )
    ST = np.ascontiguousarray(
        np.asarray(S, dtype=np.float32).T.astype(ml_dtypes.bfloat16)
    )
    TH = np.ascontiguousarray(
        np.asarray(theta, dtype=np.float32).reshape(KT, 128).T
    )
    return [
        {
            "XT": np.ascontiguousarray(
                X[i * B_CORE : (i + 1) * B_CORE].T.astype(ml_dtypes.bfloat16)
            ),
            "WeT": WeT,
            "ST": ST,
            "NTH": np.ascontiguousarray(-TH),
            "TH": TH,
        }
        for i in range(N_CORES)
    ]


def gather_out(results):
    return np.concatenate(
        [np.ascontiguousarray(results[i]["Z"].T) for i in range(N_CORES)],
        axis=0,
    ).astype(np.float32, copy=False)


def run(X, We, S, theta, trace=False, **trace_kwargs):
    nc = _get_nc()
    in_maps = make_in_maps(X, We, S, theta)
    # The PJRT compile callback can fail transiently ("CallFunctionObjArgs");
    # a retry in the same process succeeds.
    last_err = None
    for _attempt in range(3):
        try:
            res = bass_utils.run_bass_kernel_spmd(
                nc, in_maps, list(range(N_CORES)), trace=trace, **trace_kwargs
            )
            break
        except Exception as e:  # noqa: BLE001
            last_err = e
            time.sleep(2.0)
    else:
        raise last_err
    return gather_out(res.results), res


def kernel(X, We, S, theta):
    Z, _ = run(X, We, S, theta, trace=False)
    return Z


# revision 25
# speedup vs baseline: 1.0068x; 1.0014x over previous
"""Trainium2 Bass kernel for LISTA (nn_LISTA_37976100831401).

Data-parallel sharding: batch 16384 -> 8 NeuronCores x 2048 rows.
We / S / theta are replicated on every core; no cross-device comm.

Per-core algorithm (all in the transposed [feature, batch] orientation):
  B.T  = We @ X.T                  (1024, 2048)
  Z0 = soft(B);  Z_{t+1} = soft(B + Z_t @ S.T), t = 0..15
  soft(x) = relu(x - theta) - relu(-x - theta) = x - clip(x, -theta, theta)

All matmul operands are bf16: the PE streams 1 column/cycle for both
bf16 and fp32r, but bf16 weights get fast-weight-load plus
background-buffer overlap, so the per-matmul LDWEIGHTS cost disappears
(fp32r measured 227ns vs 216ns per matmul issue gap).  Accumulation
stays fp32 in PSUM; intermediates (C, relu pair) and the stored Z/B
tiles are bf16, which halves DVE/ACT data movement and doubles DVE
throughput on the all-16-bit combine.  Validated end-to-end error
~0.006-0.008 * max|expected| vs the 2e-2 gate.

Engine balance per steady-state step tile (PE budget 8 matmuls ~1.73us):
  DVE:  ct = psum + B (bf16 out), z = af - df (16-bit, 2x rate)  ~1.0us
  ACT:  af = relu(ct - th), df = relu(-ct - th)                  ~1.1us
An all-DVE soft-threshold measured DVE at 95% busy and stalled the PE.

The B phase is the tricky part: its matmul groups are only K=256 (2
matmuls, 0.43us/tile) while the per-tile threshold+copy work is
~1.6us, so a straight b(0..3) emission stalls the PE for tens of us
(measured 49us at HAM K=4/8).  Fix: alternate the b-phase threshold
between the DVE-clip path and the ACT-relu path per tile, and pipeline
the b phases into the first step sweep:
  b0 b1 s1(0) b2 s1(1) b3 s1(2) s1(3) s2(0) ...
so the PE always has step matmuls to run while b-phase chains drain.

Layout: Z.T keeps the feature dim m=1024 on SBUF partitions (8 tiles
of 128) and batch on the free dim; C.T = S @ Z.T + B.T accumulates in
PSUM via psum[j,b] += ST[k][:, j128].T @ ZT[k][:, b] and the matmul
OUTPUT layout [j, b] equals the INPUT layout [k, b] of the next step,
so no transposes anywhere.  The last step stays in [j, b] too and DMAs
Z.T out; the host transposes (host marshaling is not on the device
critical path, same as the input-side X.T/We.T/S.T preparation).
"""

import time
from contextlib import ExitStack

import ml_dtypes
import numpy as np

import concourse.bacc as bacc
import concourse.mybir as mybir
import concourse.tile as tile
from concourse import bass_utils

FP32 = mybir.dt.float32
BF16 = mybir.dt.bfloat16
AL = mybir.AluOpType
AF = mybir.ActivationFunctionType

N_CORES = 8
B_TOTAL, N_IN, M = 16384, 256, 1024
B_CORE = B_TOTAL // N_CORES  # 2048
T_STEPS = 16                 # scan length in the reference
CHUNK = 512                  # batch columns per j-sweep (= PSUM bank cap)
N_CHUNKS = B_CORE // CHUNK   # 4
KT = M // 128                # 8 feature tiles of 128
NT = N_IN // 128             # 2 input-feature tiles


def _emit(ctx: ExitStack, tc: tile.TileContext, XT, WeT, ST, NTH, TH, Z):
    nc = tc.nc

    const_pool = ctx.enter_context(tc.tile_pool(name="const", bufs=1))
    psum_pool = ctx.enter_context(tc.tile_pool(name="psum", bufs=1, space="PSUM"))
    xt_pool = ctx.enter_context(tc.tile_pool(name="xt", bufs=1))
    bt_pool = ctx.enter_context(tc.tile_pool(name="bt", bufs=1))
    zt_pool = ctx.enter_context(tc.tile_pool(name="zt", bufs=1))
    tmp_pool = ctx.enter_context(tc.tile_pool(name="tmp", bufs=1))
    out_pool = ctx.enter_context(tc.tile_pool(name="zout", bufs=1))

    # ---- constants -------------------------------------------------------
    nth = const_pool.tile([128, KT], FP32, name="nth")
    th = const_pool.tile([128, KT], FP32, name="th")
    wet = [const_pool.tile([128, M], BF16, name=f"wet{nt}") for nt in range(NT)]
    st = [const_pool.tile([128, M], BF16, name=f"st{kt}") for kt in range(KT)]

    # ---- per-chunk state -------------------------------------------------
    xts = {}  # chunk -> [NT] tiles [128, CHUNK]   (X.T slab, bf16)
    bts = {}  # chunk -> [KT] tiles [128, CHUNK]   (B.T slab, bf16)
    zts = {}  # chunk -> [KT] tiles [128, CHUNK]   (current Z.T, bf16)

    def x_phase(c, dma_eng):
        xts[c] = [
            xt_pool.tile([128, CHUNK], BF16, name=f"xt{nt}", tag=f"xt{nt}", bufs=4)
            for nt in range(NT)
        ]
        for nt in range(NT):
            dma_eng.dma_start(
                xts[c][nt][:],
                XT[nt * 128 : (nt + 1) * 128, c * CHUNK : (c + 1) * CHUNK],
            )

    def emit_head_dmas():
        # Head critical path: the b phases need wet + xt(0..3) first; S.T
        # row-blocks are needed by step(0) (~19us in): spread over three
        # queues right behind the critical loads.
        nc.scalar.dma_start(nth[:], NTH)
        nc.scalar.dma_start(th[:], TH)
        for nt in range(NT):
            nc.sync.dma_start(wet[nt][:], WeT[nt * 128 : (nt + 1) * 128, :])
        x_phase(0, nc.scalar)
        x_phase(1, nc.gpsimd)
        engs = [nc.scalar, nc.sync, nc.gpsimd]
        for kt in range(KT):
            engs[kt % 3].dma_start(st[kt][:], ST[kt * 128 : (kt + 1) * 128, :])
        x_phase(2, nc.sync)
        x_phase(3, nc.gpsimd)

    def thresh_act(jt, src, out_dtype, out_pool_, out_tag, out_bufs, io_dtype):
        # relu pair on ACT, combine on DVE (2x rate when io_dtype is bf16)
        af = tmp_pool.tile([128, CHUNK], io_dtype, name="af", tag="af", bufs=3)
        nc.scalar.activation(
            af[:], src[:], AF.Relu, bias=nth[:, jt : jt + 1], scale=1.0
        )
        df = tmp_pool.tile([128, CHUNK], io_dtype, name="df", tag="df", bufs=3)
        nc.scalar.activation(
            df[:], src[:], AF.Relu, bias=nth[:, jt : jt + 1], scale=-1.0
        )
        z = out_pool_.tile(
            [128, CHUNK], out_dtype, name="z", tag=out_tag, bufs=out_bufs
        )
        nc.vector.tensor_sub(z[:], af[:], df[:])
        return z

    def thresh_dve(jt, src, out_dtype, out_pool_, out_tag, out_bufs):
        # clip + combine, both on DVE
        cl = tmp_pool.tile([128, CHUNK], FP32, name="cl", tag="cl", bufs=3)
        nc.vector.tensor_scalar(
            cl[:], src[:], th[:, jt : jt + 1], nth[:, jt : jt + 1],
            op0=AL.min, op1=AL.max,
        )
        z = out_pool_.tile(
            [128, CHUNK], out_dtype, name="z", tag=out_tag, bufs=out_bufs
        )
        nc.vector.tensor_sub(z[:], src[:], cl[:])
        return z

    def b_phase(c):
        # B.T = We @ X.T ; Z0 = soft(B), threshold straight from PSUM.
        # PE work here is light (2-matmul groups), so the threshold load
        # alternates DVE/ACT per tile and the B copies mostly ride ACT.
        bts[c] = []
        zts[c] = []
        for jt in range(KT):
            ps = psum_pool.tile([128, CHUNK], FP32, name="psb", tag="mm", bufs=6)
            for nt in range(NT):
                nc.tensor.matmul(
                    ps[:],
                    wet[nt][:, jt * 128 : (jt + 1) * 128],
                    xts[c][nt][:],
                    start=(nt == 0),
                    stop=(nt == NT - 1),
                )
            if jt % 2 == 0:
                z0 = thresh_act(jt, ps, BF16, zt_pool, f"zt{jt}", 6, BF16)
            else:
                z0 = thresh_dve(jt, ps, BF16, zt_pool, f"zt{jt}", 6)
            btile = bt_pool.tile(
                [128, CHUNK], BF16, name="btile", tag=f"bt{jt}", bufs=4
            )
            if jt % 4 == 1:
                nc.vector.tensor_copy(btile[:], ps[:])
            else:
                nc.scalar.copy(btile[:], ps[:])
            bts[c].append(btile)
            zts[c].append(z0)

    def step(c, final=False):
        # Z <- soft(B + Z @ S.T), in the [j, b] orientation.
        zcur = zts[c]
        znew = []
        for jt in range(KT):
            ps = psum_pool.tile([128, CHUNK], FP32, name="pss", tag="mm", bufs=6)
            for kt in range(KT):
                nc.tensor.matmul(
                    ps[:],
                    st[kt][:, jt * 128 : (jt + 1) * 128],
                    zcur[kt][:],
                    start=(kt == 0),
                    stop=(kt == KT - 1),
                )
            ct = tmp_pool.tile(
                [128, CHUNK], FP32 if final else BF16, name="ct", tag="ct", bufs=3
            )
            nc.vector.tensor_add(ct[:], ps[:], bts[c][jt][:])
            if final:
                zo = thresh_act(jt, ct, FP32, out_pool, "zo", 3, FP32)
                dma_eng = nc.sync if jt % 2 == 0 else nc.scalar
                dma_eng.dma_start(
                    Z[jt * 128 : (jt + 1) * 128, c * CHUNK : (c + 1) * CHUNK],
                    zo[:],
                )
            else:
                zn = thresh_act(jt, ct, BF16, zt_pool, f"zt{jt}", 6, BF16)
                znew.append(zn)
        zts[c] = znew

    emit_head_dmas()
    b_phase(0)
    b_phase(1)
    step(0)
    b_phase(2)
    step(1)
    b_phase(3)
    step(2)
    step(3)
    for _ in range(T_STEPS - 2):
        for c in range(N_CHUNKS):
            step(c)
    for c in range(N_CHUNKS):
        step(c, final=True)


def build_nc():
    nc = bacc.Bacc("TRN2", target_bir_lowering=False, debug=False)
    XT = nc.dram_tensor("XT", [N_IN, B_CORE], BF16, kind="ExternalInput")
    WeT = nc.dram_tensor("WeT", [N_IN, M], BF16, kind="ExternalInput")
    ST = nc.dram_tensor("ST", [M, M], BF16, kind="ExternalInput")
    NTH = nc.dram_tensor("NTH", [128, KT], FP32, kind="ExternalInput")
    TH = nc.dram_tensor("TH", [128, KT], FP32, kind="ExternalInput")
    Z = nc.dram_tensor("Z", [M, B_CORE], FP32, kind="ExternalOutput")
    with tile.TileContext(nc) as tc:
        with ExitStack() as ctx:
            _emit(ctx, tc, XT.ap(), WeT.ap(), ST.ap(), NTH.ap(), TH.ap(), Z.ap())
    nc.compile()
    return nc


_NC_CACHE = None


def _get_nc():
    global _NC_CACHE
    if _NC_CACHE is None:
        _NC_CACHE = build_nc()
    return _NC_CACHE


def make_in_maps(X, We, S, theta):
    X = np.asarray(X, dtype=np.float32)
    WeT = np.ascontiguousarray(
        np.asarray(We, dtype=np.float32).T.astype(ml_dtypes.bfloat16)
    )
    ST = np.ascontiguousarray(
        np.asarray(S, dtype=np.float32).T.astype(ml_dtypes.bfloat16)
    )
    TH = np.ascontiguousarray(
        np.asarray(theta, dtype=np.float32).reshape(KT, 128).T
    )
    return [
        {
            "XT": np.ascontiguousarray(
                X[i * B_CORE : (i + 1) * B_CORE].T.astype(ml_dtypes.bfloat16)
            ),
            "WeT": WeT,
            "ST": ST,
            "NTH": np.ascontiguousarray(-TH),
            "TH": TH,
        }
        for i in range(N_CORES)
    ]


def gather_out(results):
    return np.concatenate(
        [np.ascontiguousarray(results[i]["Z"].T) for i in range(N_CORES)],
        axis=0,
    ).astype(np.float32, copy=False)


def run(X, We, S, theta, trace=False, **trace_kwargs):
    nc = _get_nc()
    in_maps = make_in_maps(X, We, S, theta)
    # The PJRT compile callback can fail transiently ("CallFunctionObjArgs");
    # a retry in the same process succeeds.
    last_err = None
    for _attempt in range(3):
        try:
            res = bass_utils.run_bass_kernel_spmd(
                nc, in_maps, list(range(N_CORES)), trace=trace, **trace_kwargs
            )
            break
        except Exception as e:  # noqa: BLE001
            last_err = e
            time.sleep(2.0)
    else:
        raise last_err
    return gather_out(res.results), res


def kernel(X, We, S, theta):
    Z, _ = run(X, We, S, theta, trace=False)
    return Z
